# revision 9
# baseline (speedup 1.0000x reference)
"""Trainium2 Bass kernel: 4D convolution (kernel 3^4, stride 1, pad 1) + bias.

  out[b,o,t,d,h,w] = bias[o] +
      sum_{i,at,ad,ah,aw} x[b,i,t+at-1,d+ad-1,h+ah-1,w+aw-1] * W[o,i,at,ad,ah,aw]

Shapes: x [2,16,8,8,32,32], W [32,16,3,3,3,3], bias [32] -> out [2,32,8,8,32,32].

Distribution (8 cores): data-parallel over the 16 (b, t) output slices, 2
adjacent t's per core (same as the direct scheme).

Per-core algorithm: 2D (t,d)-banded implicit GEMM fused with an F(2,3)
Winograd transform along w.
  * K (contraction, 128 partitions) packs the banded conv dims:
    p = 32*jt + 8*jd + ji with t_in = t0-1+jt, d_in = 2*db-1+jd, i = 8*ih+ji.
  * M packs two output spatials + channels: m = 64*mt + 32*md + o.
  * The w-dim taps are Winograd-transformed ON THE HOST:
      TX[.., wb, jw] = Bt . x[2wb-1 : 2wb+3]   (Bt = F(2,3) data transform)
    and the weights carry G along aw:
      TW[ih,ah,jw][k, m] = sum_aw G[jw,aw] * W[o, i, jt-mt, jd-md, ah, aw].
    Products P_jw = sum_{ih,ah} TW[ih,ah,jw]^T @ TX[.., ah+h, wb, jw]
    then y(w even) = P0+P1+P2 + bias, y(w odd) = P1-P2+P3 + bias.
  * Per d-block: 4*3*2 = 24 matmul passes of N=512 (vs 36 equivalent for the
    direct scheme): PE work drops 1.5x to 96 passes x 512 cols = 49.2k
    cycles/core.
  * PSUM: 4 banks (jw) per d-block, ping-ponged across db parity (8 banks).
    ScalarE evicts each product bank to SBUF bf16 (bias folded into the jw=1
    eviction since it appears in both outputs); DVE combines them into
    even/odd output planes (4 bf16 tensor ops per db, all unit-stride 2x
    mode); output ships to HBM in parity-planar layout, de-interleaved on
    the host.

Host-side transforms (padded slab extraction, w-parity split + Bt, G-banded
weights) are pure numpy data-layout work inside kernel(); the hardware
kernel consumes them as external inputs.
"""

import numpy as np

I_C, O_C = 16, 32
B_FULL, T_FULL, D, H, W = 2, 8, 8, 32, 32
HP = H + 2  # padded h' size
WB = 16     # w output blocks (2 outputs each)
N_CORES = 8
NDB = 4          # d-blocks of 2 d_out each
NSTEP = 24       # (jw, ah, ih) K-passes
TXF = 4 * HP * WB  # free size of a TX tile: (jw, h', wb)

_NC_CACHE: list = []


def emit_conv(tc, y_d, tx_d, tw_d, bb_d):
    """Emit the per-core conv program into TileContext `tc`.

    y_d [2, 32, 8, 2, 32, 16] out (mt, o, d, parity, h, wb);
    tx_d [4, 2, 128, TXF] transformed x (db, ih, p=(jt,jd,ji), (jw,h',wb));
    tw_d [128, 24, 128] G-banded weights (k, s=(jw,ah,ih), m);
    bb_d [128] broadcast bias.
    """
    import concourse.mybir as mybir

    nc = tc.nc
    f32 = mybir.dt.float32
    bf16 = mybir.dt.bfloat16
    Ident = mybir.ActivationFunctionType.Identity
    tx_d = tx_d.bitcast(bf16)
    tw_d = tw_d.bitcast(bf16)
    y_d = y_d.bitcast(bf16)

    with (
        tc.tile_pool(name="xpool", bufs=1) as xpool,
        tc.tile_pool(name="wpool", bufs=1) as wpool,
        tc.tile_pool(name="epool", bufs=1) as epool,
        tc.tile_pool(name="ppool", bufs=1, space="PSUM") as ppool,
    ):
        # ---- PSUM accumulators: (db parity, jw) -> 8 banks ----
        acc = {}
        for par in range(2):
            for jw in range(4):
                acc[par, jw] = ppool.tile(
                    [128, 512], f32, name=f"acc{par}{jw}", tag=f"acc{par}{jw}"
                )

        # ---- warmup: keep the PE busy (and un-throttle HAM) during the
        # input-DMA lead-in.  Zero matmuls into bank (0,0); the first real
        # matmul there uses start=True, which discards these results.
        WZ = wpool.tile([128, 128], bf16, name="WZ")
        nc.vector.memset(WZ[:, :], 0.0)
        for _ in range(24):
            nc.tensor.matmul(
                out=acc[0, 0][:, 0:128],
                lhsT=WZ[:, :],
                rhs=WZ[:, :],
                start=True,
                stop=True,
            )

        # ---- SBUF tiles ----
        TW = wpool.tile([128, NSTEP * 128], bf16, name="TW")
        TWv = TW.rearrange("p (k m) -> p k m", k=NSTEP)
        BB = wpool.tile([128, 1], f32, name="BB")

        TXraw, TX = {}, {}
        for db in range(NDB):
            for ih in range(2):
                t = xpool.tile([128, TXF], bf16, name=f"TX{db}{ih}")
                TXraw[db, ih] = t
                TX[db, ih] = t.rearrange("p (j h w) -> p j h w", j=4, h=HP)

        pe = {}
        for db in range(NDB):
            for jw in range(4):
                pe[db, jw] = epool.tile([128, 512], bf16, name=f"pe{db}{jw}")
        tt = {db: epool.tile([128, 512], bf16, name=f"tt{db}") for db in range(NDB)}
        ut = {db: epool.tile([128, 512], bf16, name=f"ut{db}") for db in range(NDB)}
        ot = {db: epool.tile([128, 1024], bf16, name=f"ot{db}") for db in range(NDB)}

        # ---- DMA schedule.  The two HWDGE rings (sync + scalar) start
        # transfers ~0.7us before the SWDGE group spins up, so they carry
        # db0's gating jw0 sub-tiles (one per ih) plus its jw2/jw3 halves;
        # the SWDGE chain (gpsimd launches, striped over 16 SDMA engines at
        # ~340GB/s) interleaves weight chunks and TX tiles in the order the
        # jw-major pass stream consumes them.
        q = 2 * HP * WB // 2  # one jw plane = 544 elems
        nc.sync.dma_start(out=TXraw[0, 0][:, 0:q], in_=tx_d[0, 0, :, 0:q])
        nc.scalar.dma_start(out=TXraw[0, 1][:, 0:q], in_=tx_d[0, 1, :, 0:q])
        nc.sync.dma_start(
            out=TXraw[0, 0][:, 2 * q : TXF], in_=tx_d[0, 0, :, 2 * q : TXF]
        )
        nc.scalar.dma_start(
            out=TXraw[0, 1][:, 2 * q : TXF], in_=tx_d[0, 1, :, 2 * q : TXF]
        )
        nc.sync.dma_start(out=BB[:, :], in_=bb_d.rearrange("(p u) -> p u", u=1))

        nc.gpsimd.dma_start(out=TWv[:, 0:6, :], in_=tw_d[:, 0:6])
        nc.gpsimd.dma_start(out=TXraw[0, 0][:, q : 2 * q], in_=tx_d[0, 0, :, q : 2 * q])
        nc.gpsimd.dma_start(out=TWv[:, 6:12, :], in_=tw_d[:, 6:12])
        nc.gpsimd.dma_start(out=TXraw[0, 1][:, q : 2 * q], in_=tx_d[0, 1, :, q : 2 * q])
        nc.gpsimd.dma_start(out=TWv[:, 12:18, :], in_=tw_d[:, 12:18])
        nc.gpsimd.dma_start(out=TWv[:, 18:NSTEP, :], in_=tw_d[:, 18:NSTEP])
        for db in range(1, NDB):
            for ih in range(2):
                nc.gpsimd.dma_start(
                    out=TXraw[db, ih][:, 0 : 2 * q], in_=tx_d[db, ih, :, 0 : 2 * q]
                )
            for ih in range(2):
                nc.gpsimd.dma_start(
                    out=TXraw[db, ih][:, 2 * q : TXF], in_=tx_d[db, ih, :, 2 * q : TXF]
                )

        # ---- main loop: per d-block, jw-major passes so each product bank
        # is complete (and evictable) while later jw's matmuls stream ----
        for db in range(NDB):
            par = db % 2
            for jw in range(4):
                for ah in range(3):
                    for ih in range(2):
                        s = jw * 6 + ah * 2 + ih
                        nc.tensor.matmul(
                            out=acc[par, jw][:, :],
                            lhsT=TWv[:, s, :],
                            rhs=TX[db, ih][:, jw, ah : ah + 32, :],
                            start=(ah == 0 and ih == 0),
                            stop=(ah == 2 and ih == 1),
                        )
                # evict the finished product bank; bias rides the jw=1
                # eviction (it appears in both output parities)
                nc.scalar.activation(
                    pe[db, jw][:, :],
                    acc[par, jw][:, :],
                    Ident,
                    bias=(BB[:, :] if jw == 1 else 0.0),
                    scale=1.0,
                )
                # combine as soon as operands exist (DVE, bf16 2x mode):
                #   y_even = p0 + p1 + p2 (+b), y_odd = p1 - p2 + p3 (+b)
                if jw == 1:
                    nc.vector.tensor_add(tt[db][:, :], pe[db, 0][:, :], pe[db, 1][:, :])
                elif jw == 2:
                    nc.vector.tensor_add(
                        ot[db][:, 0:512], tt[db][:, :], pe[db, 2][:, :]
                    )
                    nc.vector.tensor_sub(
                        out=ut[db][:, :], in0=pe[db, 1][:, :], in1=pe[db, 2][:, :]
                    )
                elif jw == 3:
                    nc.vector.tensor_add(
                        ot[db][:, 512:1024], ut[db][:, :], pe[db, 3][:, :]
                    )
            # ship both parity planes: per mt one ring DMA, 2KB/partition
            # contiguous dram runs (host de-interleaves the w parity)
            for mt in range(2):
                eng = nc.sync if mt == 0 else nc.scalar
                dst = y_d[mt, 2 * db : 2 * db + 2].rearrange(
                    "d o p h w -> (d o) (p h w)"
                )
                eng.dma_start(out=dst, in_=ot[db][64 * mt : 64 * mt + 64, :])


def build_nc():
    if _NC_CACHE:
        return _NC_CACHE[0]
    import concourse.bacc as bacc
    import concourse.mybir as mybir
    from concourse.tile import TileContext

    f32 = mybir.dt.float32
    u16 = mybir.dt.uint16
    nc = bacc.Bacc("TRN2", target_bir_lowering=False, debug=False, num_devices=N_CORES)
    tx_d = nc.dram_tensor("tx", [NDB, 2, 128, TXF], u16, kind="ExternalInput").ap()
    tw_d = nc.dram_tensor("tw", [128, NSTEP, 128], u16, kind="ExternalInput").ap()
    bb_d = nc.dram_tensor("bb", [128], f32, kind="ExternalInput").ap()
    y_d = nc.dram_tensor("y", [2, D, O_C, 2, H, WB], u16, kind="ExternalOutput").ap()
    with TileContext(nc) as tc:
        emit_conv(tc, y_d, tx_d, tw_d, bb_d)
    nc.compile()
    _NC_CACHE.append(nc)
    return nc


# F(2,3) Winograd: y = At (Gg . Btd) with
#   Bt d = [x0-x2, x1+x2, x2-x1, x3-x1],  G g = [g0, (g0+g1+g2)/2, (g0-g1+g2)/2, g2]
_G = np.array(
    [[1, 0, 0], [0.5, 0.5, 0.5], [0.5, -0.5, 0.5], [0, 0, 1]], dtype=np.float32
)


def build_banded_weights(weight):
    """W [32,16,3,3,3,3] -> tw [128, 24, 128] G-banded tiles (bf16 bits).

    tw[32*jt+8*jd+ji, s=(jw,ah,ih), 64*mt+32*md+o] =
        sum_aw G[jw,aw] * W[o, 8*ih+ji, jt-mt, jd-md, ah, aw]  (0 if invalid).
    """
    tw = np.zeros((NSTEP, 128, 128), dtype=np.float32)
    for jw in range(4):
        for ah in range(3):
            for ih in range(2):
                s = jw * 6 + ah * 2 + ih
                for mt in range(2):
                    for md in range(2):
                        for at in range(3):
                            for ad in range(3):
                                jt, jd = mt + at, md + ad
                                # [ji, o] block
                                wblk = weight[:, 8 * ih : 8 * ih + 8, at, ad, ah, :]
                                tw[
                                    s,
                                    32 * jt + 8 * jd : 32 * jt + 8 * jd + 8,
                                    64 * mt + 32 * md : 64 * mt + 32 * md + 32,
                                ] += np.einsum("a,oja->jo", _G[jw], wblk)
    # partition-major [p, s, m]
    return to_bf16(np.ascontiguousarray(tw.transpose(1, 0, 2)))


def to_bf16(a):
    """fp32 -> bf16 bits (round-to-nearest-even), as uint16."""
    v = np.ascontiguousarray(a, dtype=np.float32).view(np.uint32)
    return ((v + 0x7FFF + ((v >> 16) & 1)) >> 16).astype(np.uint16)


def from_bf16(u):
    return (u.astype(np.uint32) << 16).view(np.float32)


def shard_inputs(x, weight, bias):
    """Full inputs -> per-core in_maps (w-transformed slabs, banded W, bias)."""
    x = np.ascontiguousarray(np.asarray(x, dtype=np.float32))
    weight = np.ascontiguousarray(np.asarray(weight, dtype=np.float32))
    bias = np.ascontiguousarray(np.asarray(bias, dtype=np.float32))

    tw = build_banded_weights(weight)
    bb = np.ascontiguousarray(np.tile(bias, 4))  # partition m = (mt,md,o)

    in_maps = []
    for c in range(N_CORES):
        b = c // 4
        t0 = 2 * (c % 4)
        # padded slab xp[i, jt, dpad, h', w']: t_in = t0-1+jt, d_in = dpad-1,
        # h = h'-1, w = w'-1; zeros outside the tensor.
        xp = np.zeros((I_C, 4, D + 2, HP, W + 2), dtype=np.float32)
        lo, hi = t0 - 1, t0 + 3
        slo, shi = max(lo, 0), min(hi, T_FULL)
        xp[:, slo - lo : shi - lo, 1 : 1 + D, 1 : 1 + H, 1 : 1 + W] = x[b, :, slo:shi]
        # w-parity split + F(2,3) data transform along w:
        #   block wb covers w_in = 2wb-1 .. 2wb+2 -> (x0,x1,x2,x3)
        we, wo = xp[..., 0::2], xp[..., 1::2]  # 17 each
        tx4 = np.stack(
            [
                we[..., :-1] - we[..., 1:],
                wo[..., :-1] + we[..., 1:],
                we[..., 1:] - wo[..., :-1],
                wo[..., 1:] - wo[..., :-1],
            ],
            axis=-1,
        )  # [i, jt, dpad, h', wb, jw]
        # per (db, ih): [128=(jt,jd,ji), (jw, h', wb)]
        txt = np.empty((NDB, 2, 128, TXF), dtype=np.float32)
        for db in range(NDB):
            for ih in range(2):
                blk = tx4[8 * ih : 8 * ih + 8, :, 2 * db : 2 * db + 4]
                # [ji, jt, jd, h', wb, jw] -> [(jt,jd,ji), jw, h', wb]
                txt[db, ih] = (
                    blk.transpose(1, 2, 0, 5, 3, 4).reshape(128, TXF)
                )
        in_maps.append({"tx": to_bf16(txt), "tw": tw, "bb": bb})
    return in_maps


def unshard_outputs(results):
    out = np.empty((B_FULL, O_C, T_FULL, D, H, W), dtype=np.float32)
    for c in range(N_CORES):
        b = c // 4
        t0 = 2 * (c % 4)
        y = from_bf16(results[c]["y"])  # [mt, d, o, parity, h, wb]
        for mt in range(2):
            out[b, :, t0 + mt, :, :, 0::2] = y[mt, :, :, 0].transpose(1, 0, 2, 3)
            out[b, :, t0 + mt, :, :, 1::2] = y[mt, :, :, 1].transpose(1, 0, 2, 3)
    return out


def run(inputs, trace=False, **kwargs):
    from concourse.bass_utils import run_bass_kernel_spmd

    nc = build_nc()
    in_maps = shard_inputs(inputs["x"], inputs["weight"], inputs["bias"])
    res = run_bass_kernel_spmd(
        nc, in_maps, core_ids=list(range(N_CORES)), trace=trace, **kwargs
    )
    return unshard_outputs(res.results), res


def kernel(x, weight, bias):
    out, _ = run({"x": x, "weight": weight, "bias": bias})
    return out


# revision 13
# speedup vs baseline: 1.0847x; 1.0847x over previous
"""Trainium2 Bass kernel: 4D convolution (kernel 3^4, stride 1, pad 1) + bias.

  out[b,o,t,d,h,w] = bias[o] +
      sum_{i,at,ad,ah,aw} x[b,i,t+at-1,d+ad-1,h+ah-1,w+aw-1] * W[o,i,at,ad,ah,aw]

Shapes: x [2,16,8,8,32,32], W [32,16,3,3,3,3], bias [32] -> out [2,32,8,8,32,32].

Distribution (8 cores): data-parallel over the 16 (b, t) output slices, 2
adjacent t's per core (same as the direct scheme).

Per-core algorithm: 2D (t,d)-banded implicit GEMM fused with an F(2,3)
Winograd transform along w.
  * K (contraction, 128 partitions) packs the banded conv dims:
    p = 32*jt + 8*jd + ji with t_in = t0-1+jt, d_in = 2*db-1+jd, i = 8*ih+ji.
  * M packs two output spatials + channels: m = 64*mt + 32*md + o.
  * The w-dim taps are Winograd-transformed ON THE HOST:
      TX[.., wb, jw] = Bt . x[2wb-1 : 2wb+3]   (Bt = F(2,3) data transform)
    and the weights carry G along aw:
      TW[ih,ah,jw][k, m] = sum_aw G[jw,aw] * W[o, i, jt-mt, jd-md, ah, aw].
    Products P_jw = sum_{ih,ah} TW[ih,ah,jw]^T @ TX[.., ah+h, wb, jw]
    then y(w even) = P0+P1+P2 + bias, y(w odd) = P1-P2+P3 + bias.
  * Per d-block: 4*3*2 = 24 matmul passes of N=512 (vs 36 equivalent for the
    direct scheme): PE work drops 1.5x to 96 passes x 512 cols = 49.2k
    cycles/core.
  * PSUM: 4 banks (jw) per d-block, ping-ponged across db parity (8 banks).
    ScalarE evicts each product bank to SBUF bf16 (bias folded into the jw=1
    eviction since it appears in both outputs); DVE combines them into
    even/odd output planes (4 bf16 tensor ops per db, all unit-stride 2x
    mode); output ships to HBM in parity-planar layout, de-interleaved on
    the host.

Host-side transforms (padded slab extraction, w-parity split + Bt, G-banded
weights) are pure numpy data-layout work inside kernel(); the hardware
kernel consumes them as external inputs.
"""

import numpy as np

I_C, O_C = 16, 32
B_FULL, T_FULL, D, H, W = 2, 8, 8, 32, 32
HP = H + 2  # padded h' size
WB = 16     # w output blocks (2 outputs each)
N_CORES = 8
NDB = 4          # d-blocks of 2 d_out each
NSTEP = 24       # (jw, ah, ih) K-passes
TXF = 4 * HP * WB  # free size of a TX tile: (jw, h', wb)

_NC_CACHE: list = []


def emit_conv(tc, y_d, tx_d, tw_d, bb_d):
    """Emit the per-core conv program into TileContext `tc`.

    y_d [2, 32, 8, 2, 32, 16] out (mt, o, d, parity, h, wb);
    tx_d [4, 2, 128, TXF] transformed x (db, ih, p=(jt,jd,ji), (jw,h',wb));
    tw_d [128, 24, 128] G-banded weights (k, s=(jw,ah,ih), m);
    bb_d [128] broadcast bias.
    """
    import concourse.mybir as mybir

    nc = tc.nc
    f32 = mybir.dt.float32
    bf16 = mybir.dt.bfloat16
    Ident = mybir.ActivationFunctionType.Identity
    tx_d = tx_d.bitcast(bf16)
    tw_d = tw_d.bitcast(bf16)
    y_d = y_d.bitcast(bf16)

    with (
        tc.tile_pool(name="xpool", bufs=1) as xpool,
        tc.tile_pool(name="wpool", bufs=1) as wpool,
        tc.tile_pool(name="epool", bufs=1) as epool,
        tc.tile_pool(name="ppool", bufs=1, space="PSUM") as ppool,
    ):
        # ---- PSUM accumulators: (db parity, jw) -> 8 banks ----
        acc = {}
        for par in range(2):
            for jw in range(4):
                acc[par, jw] = ppool.tile(
                    [128, 512], f32, name=f"acc{par}{jw}", tag=f"acc{par}{jw}"
                )

        # ---- warmup: keep the PE busy (and un-throttle HAM) during the
        # input-DMA lead-in.  Zero matmuls into bank (0,0); the first real
        # matmul there uses start=True, which discards these results.
        WZ = wpool.tile([128, 128], bf16, name="WZ")
        nc.vector.memset(WZ[:, :], 0.0)
        for _ in range(30):
            nc.tensor.matmul(
                out=acc[0, 1][:, 0:128],
                lhsT=WZ[:, :],
                rhs=WZ[:, :],
                start=True,
                stop=True,
            )

        # ---- SBUF tiles ----
        TW = wpool.tile([128, NSTEP * 128], bf16, name="TW")
        TWv = TW.rearrange("p (k m) -> p k m", k=NSTEP)
        BB = wpool.tile([128, 1], f32, name="BB")

        TXraw, TX = {}, {}
        for db in range(NDB):
            for ih in range(2):
                t = xpool.tile([128, TXF], bf16, name=f"TX{db}{ih}")
                TXraw[db, ih] = t
                TX[db, ih] = t.rearrange("p (j h w) -> p j h w", j=4, h=HP)

        pe = {}
        for db in range(NDB):
            for jw in range(4):
                pe[db, jw] = epool.tile([128, 512], bf16, name=f"pe{db}{jw}")
        tt = {db: epool.tile([128, 512], bf16, name=f"tt{db}") for db in range(NDB)}
        ut = {db: epool.tile([128, 512], bf16, name=f"ut{db}") for db in range(NDB)}
        ot = {db: epool.tile([128, 1024], bf16, name=f"ot{db}") for db in range(NDB)}

        # ---- DMA schedule.  All bulk loads ride the SWDGE group (gpsimd
        # launches, striped over 16 SDMA engines at ~340GB/s aggregate;
        # first bytes flow ~3us after the first launch).  Chunks are ordered
        # exactly as the (jw 1,2,3,0)-major pass stream consumes them; the
        # slow HWDGE rings only carry the tiny bias plus the pair-0 output
        # stores (hidden under compute).
        q = HP * WB  # one jw plane = 544 elems
        nc.sync.dma_start(out=BB[:, :], in_=bb_d.rearrange("(p u) -> p u", u=1))

        def txc(db, ih, j0, j1):
            nc.gpsimd.dma_start(
                out=TXraw[db, ih][:, j0 * q : j1 * q],
                in_=tx_d[db, ih, :, j0 * q : j1 * q],
            )

        nc.gpsimd.dma_start(out=TWv[:, 6:12, :], in_=tw_d[:, 6:12])  # jw1
        for ih in range(2):
            for db in (0, 1):
                txc(db, ih, 1, 2)
        nc.gpsimd.dma_start(out=TWv[:, 12:NSTEP, :], in_=tw_d[:, 12:NSTEP])  # jw2,3
        for ih in range(2):
            for db in (0, 1):
                txc(db, ih, 2, 4)
        nc.gpsimd.dma_start(out=TWv[:, 0:6, :], in_=tw_d[:, 0:6])  # jw0
        for ih in range(2):
            for db in (0, 1):
                txc(db, ih, 0, 1)
        for ih in range(2):
            for db in (2, 3):
                txc(db, ih, 1, 2)
        for ih in range(2):
            for db in (2, 3):
                txc(db, ih, 2, 4)
        for ih in range(2):
            for db in (2, 3):
                txc(db, ih, 0, 1)

        # ---- main loop: d-block PAIRS with db innermost, so each
        # LDWEIGHTS feeds two matmuls (the 24 TW tiles are db-independent);
        # the pair uses all 8 PSUM banks (2 db-parities x 4 jw).  jw order
        # (1,2,3,0): after jw3 the odd-parity output is complete and ships
        # while jw0 streams; jw0 completes the even parity.
        def ship(db, parity):
            for mt in range(2):
                if db < 2:
                    eng = nc.sync if mt == 0 else nc.scalar
                else:
                    eng = nc.gpsimd
                dst = y_d[mt, 2 * db : 2 * db + 2, :, parity].rearrange(
                    "d o h w -> (d o) (h w)"
                )
                eng.dma_start(
                    out=dst,
                    in_=ot[db][64 * mt : 64 * mt + 64, 512 * parity : 512 * parity + 512],
                )

        for pair in range(2):
            dbs = (2 * pair, 2 * pair + 1)
            for jw in (1, 2, 3, 0):
                for ih in range(2):
                    for ah in range(3):
                        s = jw * 6 + ih * 3 + ah
                        for db in dbs:
                            nc.tensor.matmul(
                                out=acc[db % 2, jw][:, :],
                                lhsT=TWv[:, s, :],
                                rhs=TX[db, ih][:, jw, ah : ah + 32, :],
                                start=(ih == 0 and ah == 0),
                                stop=(ih == 2 - 1 and ah == 2),
                            )
                # evict the finished product banks; bias rides the jw=1
                # eviction (it appears in both output parities)
                for db in dbs:
                    nc.scalar.activation(
                        pe[db, jw][:, :],
                        acc[db % 2, jw][:, :],
                        Ident,
                        bias=(BB[:, :] if jw == 1 else 0.0),
                        scale=1.0,
                    )
                # combine as operands complete (DVE, bf16 2x mode):
                #   y_even = p0 + p1 + p2 (+b), y_odd = p1 - p2 + p3 (+b)
                for db in dbs:
                    if jw == 2:
                        nc.vector.tensor_sub(
                            out=ut[db][:, :], in0=pe[db, 1][:, :], in1=pe[db, 2][:, :]
                        )
                    elif jw == 3:
                        nc.vector.tensor_add(
                            ot[db][:, 512:1024], ut[db][:, :], pe[db, 3][:, :]
                        )
                    elif jw == 0:
                        nc.vector.tensor_add(
                            tt[db][:, :], pe[db, 0][:, :], pe[db, 1][:, :]
                        )
                        nc.vector.tensor_add(
                            ot[db][:, 0:512], tt[db][:, :], pe[db, 2][:, :]
                        )
                if jw == 3:
                    for db in dbs:
                        ship(db, 1)
                elif jw == 0:
                    for db in dbs:
                        ship(db, 0)


def build_nc():
    if _NC_CACHE:
        return _NC_CACHE[0]
    import concourse.bacc as bacc
    import concourse.mybir as mybir
    from concourse.tile import TileContext

    f32 = mybir.dt.float32
    u16 = mybir.dt.uint16
    nc = bacc.Bacc("TRN2", target_bir_lowering=False, debug=False, num_devices=N_CORES)
    tx_d = nc.dram_tensor("tx", [NDB, 2, 128, TXF], u16, kind="ExternalInput").ap()
    tw_d = nc.dram_tensor("tw", [128, NSTEP, 128], u16, kind="ExternalInput").ap()
    bb_d = nc.dram_tensor("bb", [128], f32, kind="ExternalInput").ap()
    y_d = nc.dram_tensor("y", [2, D, O_C, 2, H, WB], u16, kind="ExternalOutput").ap()
    with TileContext(nc) as tc:
        emit_conv(tc, y_d, tx_d, tw_d, bb_d)
    nc.compile()
    _NC_CACHE.append(nc)
    return nc


# F(2,3) Winograd: y = At (Gg . Btd) with
#   Bt d = [x0-x2, x1+x2, x2-x1, x3-x1],  G g = [g0, (g0+g1+g2)/2, (g0-g1+g2)/2, g2]
_G = np.array(
    [[1, 0, 0], [0.5, 0.5, 0.5], [0.5, -0.5, 0.5], [0, 0, 1]], dtype=np.float32
)


def build_banded_weights(weight):
    """W [32,16,3,3,3,3] -> tw [128, 24, 128] G-banded tiles (bf16 bits).

    tw[32*jt+8*jd+ji, s=(jw,ah,ih), 64*mt+32*md+o] =
        sum_aw G[jw,aw] * W[o, 8*ih+ji, jt-mt, jd-md, ah, aw]  (0 if invalid).
    """
    tw = np.zeros((NSTEP, 128, 128), dtype=np.float32)
    for jw in range(4):
        for ah in range(3):
            for ih in range(2):
                s = jw * 6 + ih * 3 + ah
                for mt in range(2):
                    for md in range(2):
                        for at in range(3):
                            for ad in range(3):
                                jt, jd = mt + at, md + ad
                                # [ji, o] block
                                wblk = weight[:, 8 * ih : 8 * ih + 8, at, ad, ah, :]
                                tw[
                                    s,
                                    32 * jt + 8 * jd : 32 * jt + 8 * jd + 8,
                                    64 * mt + 32 * md : 64 * mt + 32 * md + 32,
                                ] += np.einsum("a,oja->jo", _G[jw], wblk)
    # partition-major [p, s, m]
    return to_bf16(np.ascontiguousarray(tw.transpose(1, 0, 2)))


def to_bf16(a):
    """fp32 -> bf16 bits (round-to-nearest-even), as uint16."""
    v = np.ascontiguousarray(a, dtype=np.float32).view(np.uint32)
    return ((v + 0x7FFF + ((v >> 16) & 1)) >> 16).astype(np.uint16)


def from_bf16(u):
    return (u.astype(np.uint32) << 16).view(np.float32)


def shard_inputs(x, weight, bias):
    """Full inputs -> per-core in_maps (w-transformed slabs, banded W, bias)."""
    x = np.ascontiguousarray(np.asarray(x, dtype=np.float32))
    weight = np.ascontiguousarray(np.asarray(weight, dtype=np.float32))
    bias = np.ascontiguousarray(np.asarray(bias, dtype=np.float32))

    tw = build_banded_weights(weight)
    bb = np.ascontiguousarray(np.tile(bias, 4))  # partition m = (mt,md,o)

    in_maps = []
    for c in range(N_CORES):
        b = c // 4
        t0 = 2 * (c % 4)
        # padded slab xp[i, jt, dpad, h', w']: t_in = t0-1+jt, d_in = dpad-1,
        # h = h'-1, w = w'-1; zeros outside the tensor.
        xp = np.zeros((I_C, 4, D + 2, HP, W + 2), dtype=np.float32)
        lo, hi = t0 - 1, t0 + 3
        slo, shi = max(lo, 0), min(hi, T_FULL)
        xp[:, slo - lo : shi - lo, 1 : 1 + D, 1 : 1 + H, 1 : 1 + W] = x[b, :, slo:shi]
        # w-parity split + F(2,3) data transform along w:
        #   block wb covers w_in = 2wb-1 .. 2wb+2 -> (x0,x1,x2,x3)
        we, wo = xp[..., 0::2], xp[..., 1::2]  # 17 each
        tx4 = np.stack(
            [
                we[..., :-1] - we[..., 1:],
                wo[..., :-1] + we[..., 1:],
                we[..., 1:] - wo[..., :-1],
                wo[..., 1:] - wo[..., :-1],
            ],
            axis=-1,
        )  # [i, jt, dpad, h', wb, jw]
        # per (db, ih): [128=(jt,jd,ji), (jw, h', wb)]
        txt = np.empty((NDB, 2, 128, TXF), dtype=np.float32)
        for db in range(NDB):
            for ih in range(2):
                blk = tx4[8 * ih : 8 * ih + 8, :, 2 * db : 2 * db + 4]
                # [ji, jt, jd, h', wb, jw] -> [(jt,jd,ji), jw, h', wb]
                txt[db, ih] = (
                    blk.transpose(1, 2, 0, 5, 3, 4).reshape(128, TXF)
                )
        in_maps.append({"tx": to_bf16(txt), "tw": tw, "bb": bb})
    return in_maps


def unshard_outputs(results):
    out = np.empty((B_FULL, O_C, T_FULL, D, H, W), dtype=np.float32)
    for c in range(N_CORES):
        b = c // 4
        t0 = 2 * (c % 4)
        y = from_bf16(results[c]["y"])  # [mt, d, o, parity, h, wb]
        for mt in range(2):
            out[b, :, t0 + mt, :, :, 0::2] = y[mt, :, :, 0].transpose(1, 0, 2, 3)
            out[b, :, t0 + mt, :, :, 1::2] = y[mt, :, :, 1].transpose(1, 0, 2, 3)
    return out


def run(inputs, trace=False, **kwargs):
    from concourse.bass_utils import run_bass_kernel_spmd

    nc = build_nc()
    in_maps = shard_inputs(inputs["x"], inputs["weight"], inputs["bias"])
    res = run_bass_kernel_spmd(
        nc, in_maps, core_ids=list(range(N_CORES)), trace=trace, **kwargs
    )
    return unshard_outputs(res.results), res


def kernel(x, weight, bias):
    out, _ = run({"x": x, "weight": weight, "bias": bias})
    return out
